# revision 2
# baseline (speedup 1.0000x reference)
"""LocalScoreMachine Trainium2 kernel, v4: fp8 DoubleRow box matmuls.

arg[b, po, n] = sum over 3 halo row-pairs of Lpair^T (x) rows-pair, where each
DoubleRow fp8 matmul contracts 2 row-ktiles (256 partitions) at 0.5 cyc/col.
Lpair matrices (band x xhat, fp8) are built on HOST and shipped (1.57MB),
removing all DVE L-builds. rows shipped fp8 (4.2MB, zero-padded border rows),
imgs f16 (6.3MB) for the DVE product+accum reductions. w = exp(arg) on ACT
(SW via accum_out); SWI_c = TT-mult (2x) + TSP accum (4x); a fraction of the
accums routes to ACT Copy+accum to balance DVE vs ACT.
Sharding: 4 query-pair groups x 2 dataset halves.
"""

import sys

for _p in ("/opt/trn_rl_repo", "/opt/trn_rl_repo/concourse", "/opt/pypackages"):
    if _p not in sys.path:
        sys.path.append(_p)

from contextlib import ExitStack

import numpy as np

import concourse.bass as bass
import concourse.bacc as bacc
import concourse.mybir as mybir
import concourse.tile as tile
from concourse import bass_utils

B, N, C, H, W = 8, 2048, 3, 32, 32
P = H * W
NCORES = 8
GB, GN = 4, 2
NB = B // GB  # 2
NLOC = N // GN  # 1024
PT = P // 128  # 8
F = NLOC
CH = 512
NPAIR = 3  # halo row-pairs per output tile

F32 = mybir.dt.float32
F16 = mybir.dt.float16
F8 = mybir.dt.float8e4
AF = mybir.ActivationFunctionType
OP = mybir.AluOpType
DR = mybir.MatmulPerfMode.DoubleRow

N_ACT = 9  # of the DVE-product accums, this many go to ACT Copy
POOL_NT = 7  # c=2 products run on Pool for t < POOL_NT (early, to avoid tail)

_cache = {}
_last_res = None


def _build():
    nc = bacc.Bacc("TRN2", target_bir_lowering=False, debug=False)

    rows_d = nc.dram_tensor("rows", [32, 128, F], F8, kind="ExternalInput")
    lstk_d = nc.dram_tensor("lstk", [128, 16 * NPAIR * 256], F8, kind="ExternalInput")
    img16_d = nc.dram_tensor("img16", [PT, 128, 2, F], F16, kind="ExternalInput")
    img8_d = nc.dram_tensor("img8", [POOL_NT, 128, F], F8, kind="ExternalInput")
    imgc2_d = nc.dram_tensor(
        "imgc2", [PT - POOL_NT, 128, F], F16, kind="ExternalInput"
    )
    out_d = nc.dram_tensor("out", [128, NB * PT * 4 + 8], F32, kind="ExternalOutput")

    with tile.TileContext(nc) as tc, ExitStack() as ctx:
        const = ctx.enter_context(tc.tile_pool(name="const", bufs=1))
        rpool = ctx.enter_context(tc.tile_pool(name="rpool", bufs=1))
        imgs = ctx.enter_context(tc.tile_pool(name="imgs", bufs=1))
        wp = ctx.enter_context(tc.tile_pool(name="wp", bufs=4))
        scr = ctx.enter_context(tc.tile_pool(name="scr", bufs=5))
        psum = ctx.enter_context(
            tc.tile_pool(name="psum", bufs=3, space=bass.MemorySpace.PSUM)
        )

        stage = const.tile([128, NB * PT * 4 + 8], F32)

        lstk = const.tile([128, 16 * NPAIR * 256], F8)
        rows = rpool.tile([128, 34 * F], F8)
        rows3 = rows[:].rearrange("p (r n) -> p r n", r=34)
        it16s, ic2s = [], []
        for g in range(PT):
            it16s.append(
                imgs.tile([128, 2 * F], F16, tag=f"img{g}", name=f"img{g}")
            )
            if g < POOL_NT:
                ic2s.append(imgs.tile([128, F], F8, tag=f"i8_{g}", name=f"i8_{g}"))
            else:
                ic2s.append(
                    imgs.tile([128, F], F16, tag=f"ic2_{g}", name=f"ic2_{g}")
                )

        # zero the border rows (0 and 33) on Pool instead of shipping them
        nc.gpsimd.memset(rows3[:, 0:1, :], 0)
        nc.gpsimd.memset(rows3[:, 33:34, :], 0)

        psz = NPAIR * 256  # lstk elems per step
        LCH = [(0, 2), (2, 4), (4, 7), (7, 12), (12, 16)]  # step ranges

        def dma_lstk(i):
            s0, s1 = LCH[i]
            nc.sync.dma_start(
                lstk[:, s0 * psz : s1 * psz], lstk_d.ap()[:, s0 * psz : s1 * psz]
            )

        # rows chunk ranges in DRAM row index r (sbuf index = r+1)
        RCH = [(0, 3), (3, 5), (5, 9), (9, 13), (13, 17), (17, 21),
               (21, 25), (25, 29), (29, 32)]

        def dma_rows(i):
            r0, r1 = RCH[i]
            nc.sync.dma_start(
                rows3[:, r0 + 1 : r1 + 1, :],
                rows_d.ap()[r0:r1].rearrange("r p n -> p r n"),
            )

        def dma_img(g):
            nc.sync.dma_start(
                it16s[g][:].rearrange("p (c n) -> p c n", c=2), img16_d.ap()[g]
            )
            if g < POOL_NT:
                nc.sync.dma_start(ic2s[g][:], img8_d.ap()[g])
            else:
                nc.sync.dma_start(ic2s[g][:], imgc2_d.ap()[g - POOL_NT])

        # single queue, earliest-deadline order: the DMA_ENGINES device
        # serializes transfers in ready order, so issue order == arrival order
        dma_lstk(0); dma_rows(0); dma_rows(1); dma_img(0)
        dma_lstk(1); dma_rows(2); dma_img(1)
        dma_lstk(2); dma_rows(3); dma_img(2)
        dma_rows(4); dma_lstk(3); dma_img(3)
        dma_rows(5); dma_img(4)
        dma_rows(6); dma_lstk(4); dma_img(5)
        dma_rows(7); dma_img(6)
        dma_rows(8); dma_img(7)

        def scol(b, t, j):
            k = b * (PT * 4) + t * 4 + j
            return stage[:, k : k + 1]

        def scol2(b, j):  # extra columns for the t=0 second-half partials
            k = NB * PT * 4 + b * 4 + j
            return stage[:, k : k + 1]

        def lpair_ap(b, t, j):
            k = ((t * NB + b) * NPAIR + j) * 256
            return lstk[:, k : k + 256].rearrange("p (two m) -> p two m", two=2)

        pst = [[None] * PT for _ in range(NB)]

        def emit_mm(b, t):
            # t=0 uses two half psum tiles so each half-exp can start as soon
            # as its own 3 matmuls finish (psum deps are tile-granular)
            if t == 0:
                hs = []
                for ck in range(F // CH):
                    ph = psum.tile(
                        [128, CH], F32, tag=f"psh{ck}", name=f"psh_{b}_{ck}",
                        bufs=1,
                    )
                    for j in range(NPAIR):
                        ri0 = 4 * t + 2 * j
                        nc.tensor.matmul(
                            ph[:],
                            lpair_ap(b, t, j),
                            rows3[:, ri0 : ri0 + 2, ck * CH : (ck + 1) * CH],
                            start=(j == 0),
                            stop=(j == NPAIR - 1),
                            perf_mode=DR,
                            skip_group_check=True,
                        )
                    hs.append(ph)
                pst[b][t] = hs
                return
            ps = psum.tile([128, F], F32, tag="ps", name=f"ps_{b}_{t}")
            for ck in range(F // CH):
                for j in range(NPAIR):
                    ri0 = 4 * t + 2 * j  # rows index of pair start (r = ri0-1)
                    nc.tensor.matmul(
                        ps[:, ck * CH : (ck + 1) * CH],
                        lpair_ap(b, t, j),
                        rows3[:, ri0 : ri0 + 2, ck * CH : (ck + 1) * CH],
                        start=(j == 0),
                        stop=(j == NPAIR - 1),
                        perf_mode=DR,
                        skip_group_check=True,
                    )
            pst[b][t] = ps

        wts = [[None] * PT for _ in range(NB)]

        def emit_exp(b, t):
            w = wp.tile([128, F], F16, tag="w", name=f"w_{b}_{t}")
            ps = pst[b][t]
            if t == 0:
                nc.scalar.activation(
                    w[:, 0:CH], ps[0][:], AF.Exp, accum_out=scol(b, t, 0)
                )
                nc.scalar.activation(
                    w[:, CH:F], ps[1][:], AF.Exp, accum_out=scol2(b, 0)
                )
            else:
                nc.scalar.activation(
                    w[:], ps[:], AF.Exp, accum_out=scol(b, t, 0)
                )
            wts[b][t] = w

        aq = [0.0]
        n_dve_accum_cands = 4 * 6

        def accum(v_slice, b, t, c, early=True):
            if early and t < 6:
                aq[0] += N_ACT / float(n_dve_accum_cands)
            if aq[0] >= 1.0:
                aq[0] -= 1.0
                d = scr.tile([128, F], F16, tag="da", name=f"d{c}_{b}_{t}")
                nc.scalar.activation(
                    d[:], v_slice, AF.Copy, accum_out=scol(b, t, 1 + c)
                )
            else:
                d = scr.tile([128, F], F16, tag="dv", name=f"d{c}_{b}_{t}")
                nc.vector.tensor_scalar(
                    d[:], v_slice, 1.0, 0.0, OP.mult, OP.add,
                    accum_out=scol(b, t, 1 + c),
                )


        pend = []

        def flush_pend():
            while pend:
                v1p, bp, tp = pend.pop(0)
                d = scr.tile([128, F], F16, tag="dv", name=f"d2_{bp}_{tp}")
                nc.vector.tensor_scalar(
                    d[:], v1p[:], 1.0, 0.0, OP.mult, OP.add,
                    accum_out=scol(bp, tp, 3),
                )

        def emit_red(b, t):
            w = wts[b][t]
            it16r = it16s[t][:].rearrange("p (c n) -> p c n", c=2)
            if t == 0:
                # two half-width passes: the first starts as soon as the
                # first half-exp lands
                v2 = scr.tile([128, 2 * F], F16, tag="v2", name=f"v2_{b}_{t}")
                v2r = v2[:].rearrange("p (c n) -> p c n", c=2)
                for h, (n0, n1) in enumerate([(0, CH), (CH, F)]):
                    nc.vector.tensor_tensor(
                        v2r[:, :, n0:n1],
                        w[:, n0:n1].unsqueeze(1).broadcast_to((128, 2, CH)),
                        it16r[:, :, n0:n1],
                        OP.mult,
                    )
                    for c in range(2):
                        col = scol(b, t, 1 + c) if h == 0 else scol2(b, 1 + c)
                        d = scr.tile(
                            [128, CH], F16, tag="dv2", name=f"e{c}_{b}_{h}"
                        )
                        nc.vector.tensor_scalar(
                            d[:], v2r[:, c, n0:n1], 1.0, 0.0, OP.mult, OP.add,
                            accum_out=col,
                        )
                v1 = scr.tile([128, F], F16, tag="vp", name=f"v1_{b}_{t}")
                nc.gpsimd.tensor_tensor(v1[:], w[:], ic2s[t][:], OP.mult)
                pend.append((v1, b, t))
                return
            # c=0,1 in one 2-wide DVE op: out[(c, n)]
            v2 = scr.tile([128, 2 * F], F16, tag="v2", name=f"v2_{b}_{t}")
            v2r = v2[:].rearrange("p (c n) -> p c n", c=2)
            nc.vector.tensor_tensor(
                v2r,
                w[:].unsqueeze(1).broadcast_to((128, 2, F)),
                it16r,
                OP.mult,
            )
            # c=2: Pool (fp8 images) for t < POOL_NT, else DVE (f16 images)
            if t < POOL_NT:
                v1 = scr.tile([128, F], F16, tag="vp", name=f"v1_{b}_{t}")
                nc.gpsimd.tensor_tensor(v1[:], w[:], ic2s[t][:], OP.mult)
                pend.append((v1, b, t))
            else:
                v1 = scr.tile([128, F], F16, tag="vp", name=f"v1_{b}_{t}")
                nc.vector.tensor_tensor(v1[:], w[:], ic2s[t][:], OP.mult)
            for c in range(2):
                accum(v2r[:, c, :], b, t, c)
            flush_pend()
            if t >= POOL_NT:
                accum(v1[:], b, t, 2)

        steps = [(t, b) for t in range(PT) for b in range(NB)]
        nst = len(steps)
        for k in range(nst + 4):
            if k < nst:
                t, b = steps[k]
                emit_mm(b, t)
            if 2 <= k < nst + 2:
                t1, b1 = steps[k - 2]
                emit_exp(b1, t1)
            if 4 <= k < nst + 4:
                t2, b2 = steps[k - 4]
                emit_red(b2, t2)

        flush_pend()
        nc.sync.dma_start(out_d.ap(), stage[:])

    nc.compile()
    return nc


def _band6():
    # b6[j][(c,px_in), po]: band for input row-offset j-1 vs output row po//32
    px_i = np.arange(128) % 32
    po = np.arange(128)
    ro, pxo = po // 32, po % 32
    b6 = np.zeros((6, 128, 128), np.float32)
    for j in range(6):
        rd = (j - 1) - ro[None, :]
        wd = px_i[:, None] - pxo[None, :]
        b6[j] = ((np.abs(rd) <= 1) & (np.abs(wd) <= 1)).astype(np.float32)
    return b6


def kernel(x, images, mu, sigma, t):
    x = np.ascontiguousarray(np.asarray(x, dtype=np.float32))
    images = np.ascontiguousarray(np.asarray(images, dtype=np.float32))
    m = float(np.asarray(mu)[int(t)])
    sig = float(np.asarray(sigma)[int(t)])
    sig2 = sig * sig

    if "nc" not in _cache:
        _cache["nc"] = _build()
    nc = _cache["nc"]

    f8 = mybir.dt.np(F8)
    imgs2 = images.reshape(N, C, P)
    spp = (-(m * m) / (2.0 * sig2)) * np.einsum("ncp,ncp->np", imgs2, imgs2)
    xx = (x.reshape(B, C, H, W) * (m / sig2)).astype(np.float32)
    b6 = _band6()

    in_maps = []
    for k in range(NCORES):
        bg, nh = k // GN, k % GN
        nsl = slice(nh * NLOC, (nh + 1) * NLOC)
        # rows[r]: [(c,px), n]; c<3 = I_c row r, c=3 = S'' row r
        rows = np.zeros((32, 4, 32, F), np.float32)
        for c in range(C):
            rows[:, c] = imgs2[nsl, c, :].T.reshape(H, 32, F)
        rows[:, 3] = spp[nsl].T.reshape(H, 32, F)
        # imgs[t]: [(4r x 32px), c, n] for the product reductions; c=0,1 f16,
        # c=2 fp8 for the Pool tiles (t < POOL_NT) and f16 for the rest
        arr16 = np.empty((PT, 128, 2, F), np.float16)
        for c in range(2):
            arr16[:, :, c, :] = imgs2[nsl, c, :].T.reshape(PT, 128, F)
        c2 = imgs2[nsl, 2, :].T.reshape(PT, 128, F)
        arr8 = c2[:POOL_NT].astype(f8)
        arrc2 = c2[POOL_NT:].astype(np.float16)
        # L pair stacks: [(c,pxi), (t,b,j,pair,po)] fp8
        lstk = np.zeros((16 * NPAIR, 2, 128, 128), np.float32)  # [k,pair,pi,po]
        for tt in range(PT):
            for b in range(NB):
                gbq = bg * NB + b
                for j in range(NPAIR):
                    kk = (tt * NB + b) * NPAIR + j
                    for i in range(2):
                        r = 4 * tt - 1 + 2 * j + i
                        if not (0 <= r < H):
                            continue
                        # xsc[(c,pxi)] = xhat[gbq, c, r, pxi]; 1.0 for c=3
                        xsc = np.ones((4, 32), np.float32)
                        for c in range(C):
                            xsc[c] = xx[gbq, c, r]
                        lstk[kk, i] = b6[2 * j + i] * xsc.reshape(128)[:, None]
        # -> [pi, k, pair, po]
        lstk = np.ascontiguousarray(lstk.transpose(2, 0, 1, 3)).reshape(128, -1)
        in_maps.append(
            {
                "rows": np.ascontiguousarray(rows.reshape(32, 128, F)).astype(f8),
                "lstk": lstk.astype(f8),
                "img16": np.ascontiguousarray(arr16),
                "img8": np.ascontiguousarray(arr8),
                "imgc2": np.ascontiguousarray(arrc2),
            }
        )

    import os

    trace = bool(os.environ.get("KERNEL_TRACE"))
    res = bass_utils.run_bass_kernel_spmd(
        nc, in_maps, core_ids=list(range(NCORES)), trace=trace
    )
    global _last_res
    _last_res = res

    sw = np.zeros((B, P), np.float64)
    swi = np.zeros((B, C, P), np.float64)
    for k in range(NCORES):
        bg = k // GN
        raw = np.asarray(res.results[k]["out"], np.float64)
        st = raw[:, : NB * PT * 4].reshape(128, NB, PT, 4)
        ex = raw[:, NB * PT * 4 :].reshape(128, NB, 4)
        st[:, :, 0, 0:3] += ex[:, :, 0:3]  # col 3 (c2) is never split
        for b in range(NB):
            gb = bg * NB + b
            sw[gb] += st[:, b, :, 0].T.reshape(P)
            for c in range(C):
                swi[gb, c] += st[:, b, :, 1 + c].T.reshape(P)

    score = (m * swi / sw[:, None, :] - x.reshape(B, C, P)) / sig2
    return score.reshape(B, C, H, W).astype(np.float32)


# revision 3
# speedup vs baseline: 1.0034x; 1.0034x over previous
"""LocalScoreMachine Trainium2 kernel, v4: fp8 DoubleRow box matmuls.

arg[b, po, n] = sum over 3 halo row-pairs of Lpair^T (x) rows-pair, where each
DoubleRow fp8 matmul contracts 2 row-ktiles (256 partitions) at 0.5 cyc/col.
Lpair matrices (band x xhat, fp8) are built on HOST and shipped (1.57MB),
removing all DVE L-builds. rows shipped fp8 (4.2MB, zero-padded border rows),
imgs f16 (6.3MB) for the DVE product+accum reductions. w = exp(arg) on ACT
(SW via accum_out); SWI_c = TT-mult (2x) + TSP accum (4x); a fraction of the
accums routes to ACT Copy+accum to balance DVE vs ACT.
Sharding: 4 query-pair groups x 2 dataset halves.
"""

import sys

for _p in ("/opt/trn_rl_repo", "/opt/trn_rl_repo/concourse", "/opt/pypackages"):
    if _p not in sys.path:
        sys.path.append(_p)

from contextlib import ExitStack

import numpy as np

import concourse.bass as bass
import concourse.bacc as bacc
import concourse.mybir as mybir
import concourse.tile as tile
from concourse import bass_utils

B, N, C, H, W = 8, 2048, 3, 32, 32
P = H * W
NCORES = 8
GB, GN = 4, 2
NB = B // GB  # 2
NLOC = N // GN  # 1024
PT = P // 128  # 8
F = NLOC
CH = 512
NPAIR = 3  # halo row-pairs per output tile

F32 = mybir.dt.float32
F16 = mybir.dt.float16
F8 = mybir.dt.float8e4
AF = mybir.ActivationFunctionType
OP = mybir.AluOpType
DR = mybir.MatmulPerfMode.DoubleRow

N_ACT = 9  # of the DVE-product accums, this many go to ACT Copy
POOL_NT = 7  # c=2 products run on Pool for t < POOL_NT (early, to avoid tail)

_cache = {}
_last_res = None


def _build():
    nc = bacc.Bacc("TRN2", target_bir_lowering=False, debug=False)

    rows_d = nc.dram_tensor("rows", [32, 128, F], F8, kind="ExternalInput")
    lstk_d = nc.dram_tensor("lstk", [128, 16 * NPAIR * 256], F8, kind="ExternalInput")
    img16_d = nc.dram_tensor("img16", [PT, 128, 2, F], F16, kind="ExternalInput")
    img8_d = nc.dram_tensor("img8", [POOL_NT, 128, F], F8, kind="ExternalInput")
    imgc2_d = nc.dram_tensor(
        "imgc2", [PT - POOL_NT, 128, F], F16, kind="ExternalInput"
    )
    out_d = nc.dram_tensor("out", [128, NB * PT * 4 + 8], F32, kind="ExternalOutput")

    with tile.TileContext(nc) as tc, ExitStack() as ctx:
        const = ctx.enter_context(tc.tile_pool(name="const", bufs=1))
        rpool = ctx.enter_context(tc.tile_pool(name="rpool", bufs=1))
        imgs = ctx.enter_context(tc.tile_pool(name="imgs", bufs=1))
        wp = ctx.enter_context(tc.tile_pool(name="wp", bufs=4))
        scr = ctx.enter_context(tc.tile_pool(name="scr", bufs=5))
        psum = ctx.enter_context(
            tc.tile_pool(name="psum", bufs=3, space=bass.MemorySpace.PSUM)
        )

        stage = const.tile([128, NB * PT * 4 + 8], F32)

        lstk = const.tile([128, 16 * NPAIR * 256], F8)
        rows = rpool.tile([128, 34 * F], F8)
        rows3 = rows[:].rearrange("p (r n) -> p r n", r=34)
        it16s, ic2s = [], []
        for g in range(PT):
            it16s.append(
                imgs.tile([128, 2 * F], F16, tag=f"img{g}", name=f"img{g}")
            )
            if g < POOL_NT:
                ic2s.append(imgs.tile([128, F], F8, tag=f"i8_{g}", name=f"i8_{g}"))
            else:
                ic2s.append(
                    imgs.tile([128, F], F16, tag=f"ic2_{g}", name=f"ic2_{g}")
                )

        # zero the border rows (0 and 33) on Pool instead of shipping them
        nc.gpsimd.memset(rows3[:, 0:1, :], 0)
        nc.gpsimd.memset(rows3[:, 33:34, :], 0)

        psz = NPAIR * 256  # lstk elems per step
        LCH = [(0, 2), (2, 4), (4, 7), (7, 12), (12, 16)]  # step ranges

        def dma_lstk(i):
            s0, s1 = LCH[i]
            nc.sync.dma_start(
                lstk[:, s0 * psz : s1 * psz], lstk_d.ap()[:, s0 * psz : s1 * psz]
            )

        # rows chunk ranges in DRAM row index r (sbuf index = r+1)
        RCH = [(0, 3), (3, 5), (5, 9), (9, 13), (13, 17), (17, 21),
               (21, 25), (25, 29), (29, 32)]

        def dma_rows(i):
            r0, r1 = RCH[i]
            nc.sync.dma_start(
                rows3[:, r0 + 1 : r1 + 1, :],
                rows_d.ap()[r0:r1].rearrange("r p n -> p r n"),
            )

        def dma_img(g):
            nc.sync.dma_start(
                it16s[g][:].rearrange("p (c n) -> p c n", c=2), img16_d.ap()[g]
            )
            if g < POOL_NT:
                nc.sync.dma_start(ic2s[g][:], img8_d.ap()[g])
            else:
                nc.sync.dma_start(ic2s[g][:], imgc2_d.ap()[g - POOL_NT])

        # single queue, earliest-deadline order: the DMA_ENGINES device
        # serializes transfers in ready order, so issue order == arrival order
        dma_lstk(0); dma_rows(0); dma_rows(1); dma_img(0)
        dma_lstk(1); dma_rows(2); dma_img(1)
        dma_lstk(2); dma_rows(3); dma_img(2)
        dma_rows(4); dma_lstk(3); dma_img(3)
        dma_rows(5); dma_img(4)
        dma_rows(6); dma_lstk(4); dma_img(5)
        dma_rows(7); dma_img(6)
        dma_rows(8); dma_img(7)

        def scol(b, t, j):
            k = b * (PT * 4) + t * 4 + j
            return stage[:, k : k + 1]

        def scol2(b, j):  # extra columns for the t=0 second-half partials
            k = NB * PT * 4 + b * 4 + j
            return stage[:, k : k + 1]

        def lpair_ap(b, t, j):
            k = ((t * NB + b) * NPAIR + j) * 256
            return lstk[:, k : k + 256].rearrange("p (two m) -> p two m", two=2)

        pst = [[None] * PT for _ in range(NB)]

        def emit_mm(b, t):
            # t=0 uses two half psum tiles so each half-exp can start as soon
            # as its own 3 matmuls finish (psum deps are tile-granular)
            if t == 0:
                hs = []
                for ck in range(F // CH):
                    ph = psum.tile(
                        [128, CH], F32, tag=f"psh{ck}", name=f"psh_{b}_{ck}",
                        bufs=1,
                    )
                    for j in range(NPAIR):
                        ri0 = 4 * t + 2 * j
                        nc.tensor.matmul(
                            ph[:],
                            lpair_ap(b, t, j),
                            rows3[:, ri0 : ri0 + 2, ck * CH : (ck + 1) * CH],
                            start=(j == 0),
                            stop=(j == NPAIR - 1),
                            perf_mode=DR,
                            skip_group_check=True,
                        )
                    hs.append(ph)
                pst[b][t] = hs
                return
            ps = psum.tile([128, F], F32, tag="ps", name=f"ps_{b}_{t}")
            for ck in range(F // CH):
                for j in range(NPAIR):
                    ri0 = 4 * t + 2 * j  # rows index of pair start (r = ri0-1)
                    nc.tensor.matmul(
                        ps[:, ck * CH : (ck + 1) * CH],
                        lpair_ap(b, t, j),
                        rows3[:, ri0 : ri0 + 2, ck * CH : (ck + 1) * CH],
                        start=(j == 0),
                        stop=(j == NPAIR - 1),
                        perf_mode=DR,
                        skip_group_check=True,
                    )
            pst[b][t] = ps

        wts = [[None] * PT for _ in range(NB)]

        def emit_exp(b, t):
            w = wp.tile([128, F], F16, tag="w", name=f"w_{b}_{t}")
            ps = pst[b][t]
            if t == 0:
                nc.scalar.activation(
                    w[:, 0:CH], ps[0][:], AF.Exp, accum_out=scol(b, t, 0)
                )
                nc.scalar.activation(
                    w[:, CH:F], ps[1][:], AF.Exp, accum_out=scol2(b, 0)
                )
            else:
                nc.scalar.activation(
                    w[:], ps[:], AF.Exp, accum_out=scol(b, t, 0)
                )
            wts[b][t] = w

        aq = [0.0]
        n_dve_accum_cands = 4 * 6

        def accum(v_slice, b, t, c, early=True):
            if early and t < 6:
                aq[0] += N_ACT / float(n_dve_accum_cands)
            if aq[0] >= 1.0:
                aq[0] -= 1.0
                d = scr.tile([128, F], F16, tag="da", name=f"d{c}_{b}_{t}")
                nc.scalar.activation(
                    d[:], v_slice, AF.Copy, accum_out=scol(b, t, 1 + c)
                )
            else:
                d = scr.tile([128, F], F16, tag="dv", name=f"d{c}_{b}_{t}")
                nc.vector.tensor_scalar(
                    d[:], v_slice, 1.0, 0.0, OP.mult, OP.add,
                    accum_out=scol(b, t, 1 + c),
                )


        pend = []

        def flush_pend(keep=0):
            while len(pend) > keep:
                v1p, bp, tp = pend.pop(0)
                d = scr.tile([128, F], F16, tag="dv", name=f"d2_{bp}_{tp}")
                nc.vector.tensor_scalar(
                    d[:], v1p[:], 1.0, 0.0, OP.mult, OP.add,
                    accum_out=scol(bp, tp, 3),
                )

        def emit_red(b, t):
            w = wts[b][t]
            it16r = it16s[t][:].rearrange("p (c n) -> p c n", c=2)
            if t == 0:
                # two half-width passes: the first starts as soon as the
                # first half-exp lands
                v2 = scr.tile([128, 2 * F], F16, tag="v2", name=f"v2_{b}_{t}")
                v2r = v2[:].rearrange("p (c n) -> p c n", c=2)
                for h, (n0, n1) in enumerate([(0, CH), (CH, F)]):
                    nc.vector.tensor_tensor(
                        v2r[:, :, n0:n1],
                        w[:, n0:n1].unsqueeze(1).broadcast_to((128, 2, CH)),
                        it16r[:, :, n0:n1],
                        OP.mult,
                    )
                    for c in range(2):
                        col = scol(b, t, 1 + c) if h == 0 else scol2(b, 1 + c)
                        d = scr.tile(
                            [128, CH], F16, tag="dv2", name=f"e{c}_{b}_{h}"
                        )
                        nc.vector.tensor_scalar(
                            d[:], v2r[:, c, n0:n1], 1.0, 0.0, OP.mult, OP.add,
                            accum_out=col,
                        )
                v1 = scr.tile([128, F], F16, tag="vp", name=f"v1_{b}_{t}")
                nc.gpsimd.tensor_tensor(v1[:], w[:], ic2s[t][:], OP.mult)
                pend.append((v1, b, t))
                return
            # c=0,1 in one 2-wide DVE op: out[(c, n)]
            v2 = scr.tile([128, 2 * F], F16, tag="v2", name=f"v2_{b}_{t}")
            v2r = v2[:].rearrange("p (c n) -> p c n", c=2)
            nc.vector.tensor_tensor(
                v2r,
                w[:].unsqueeze(1).broadcast_to((128, 2, F)),
                it16r,
                OP.mult,
            )
            # c=2: Pool (fp8 images) for t < POOL_NT, else DVE (f16 images);
            # the final step stays on DVE so the tail isn't gated on Pool
            if t < POOL_NT or (t == PT - 1 and b == 0):
                v1 = scr.tile([128, F], F16, tag="vp", name=f"v1_{b}_{t}")
                nc.gpsimd.tensor_tensor(v1[:], w[:], ic2s[t][:], OP.mult)
                pend.append((v1, b, t))
            else:
                v1 = scr.tile([128, F], F16, tag="vp", name=f"v1_{b}_{t}")
                nc.vector.tensor_tensor(v1[:], w[:], ic2s[t][:], OP.mult)
            for c in range(2):
                if t == PT - 1 and b == NB - 1 and c == 0:
                    d = scr.tile([128, F], F16, tag="da", name=f"dl{c}")
                    nc.scalar.activation(
                        d[:], v2r[:, c, :], AF.Copy, accum_out=scol(b, t, 1 + c)
                    )
                else:
                    accum(v2r[:, c, :], b, t, c)
            flush_pend(keep=1)
            if t >= POOL_NT:
                accum(v1[:], b, t, 2)

        steps = [(t, b) for t in range(PT) for b in range(NB)]
        nst = len(steps)
        for k in range(nst + 4):
            if k < nst:
                t, b = steps[k]
                emit_mm(b, t)
            if 2 <= k < nst + 2:
                t1, b1 = steps[k - 2]
                emit_exp(b1, t1)
            if 4 <= k < nst + 4:
                t2, b2 = steps[k - 4]
                emit_red(b2, t2)

        flush_pend()
        nc.sync.dma_start(out_d.ap(), stage[:])

    nc.compile()
    return nc


def _band6():
    # b6[j][(c,px_in), po]: band for input row-offset j-1 vs output row po//32
    px_i = np.arange(128) % 32
    po = np.arange(128)
    ro, pxo = po // 32, po % 32
    b6 = np.zeros((6, 128, 128), np.float32)
    for j in range(6):
        rd = (j - 1) - ro[None, :]
        wd = px_i[:, None] - pxo[None, :]
        b6[j] = ((np.abs(rd) <= 1) & (np.abs(wd) <= 1)).astype(np.float32)
    return b6


def kernel(x, images, mu, sigma, t):
    x = np.ascontiguousarray(np.asarray(x, dtype=np.float32))
    images = np.ascontiguousarray(np.asarray(images, dtype=np.float32))
    m = float(np.asarray(mu)[int(t)])
    sig = float(np.asarray(sigma)[int(t)])
    sig2 = sig * sig

    if "nc" not in _cache:
        _cache["nc"] = _build()
    nc = _cache["nc"]

    f8 = mybir.dt.np(F8)
    imgs2 = images.reshape(N, C, P)
    spp = (-(m * m) / (2.0 * sig2)) * np.einsum("ncp,ncp->np", imgs2, imgs2)
    xx = (x.reshape(B, C, H, W) * (m / sig2)).astype(np.float32)
    b6 = _band6()

    in_maps = []
    for k in range(NCORES):
        bg, nh = k // GN, k % GN
        nsl = slice(nh * NLOC, (nh + 1) * NLOC)
        # rows[r]: [(c,px), n]; c<3 = I_c row r, c=3 = S'' row r
        rows = np.zeros((32, 4, 32, F), np.float32)
        for c in range(C):
            rows[:, c] = imgs2[nsl, c, :].T.reshape(H, 32, F)
        rows[:, 3] = spp[nsl].T.reshape(H, 32, F)
        # imgs[t]: [(4r x 32px), c, n] for the product reductions; c=0,1 f16,
        # c=2 fp8 for the Pool tiles (t < POOL_NT) and f16 for the rest
        arr16 = np.empty((PT, 128, 2, F), np.float16)
        for c in range(2):
            arr16[:, :, c, :] = imgs2[nsl, c, :].T.reshape(PT, 128, F)
        c2 = imgs2[nsl, 2, :].T.reshape(PT, 128, F)
        arr8 = c2[:POOL_NT].astype(f8)
        arrc2 = c2[POOL_NT:].astype(np.float16)
        # L pair stacks: [(c,pxi), (t,b,j,pair,po)] fp8
        lstk = np.zeros((16 * NPAIR, 2, 128, 128), np.float32)  # [k,pair,pi,po]
        for tt in range(PT):
            for b in range(NB):
                gbq = bg * NB + b
                for j in range(NPAIR):
                    kk = (tt * NB + b) * NPAIR + j
                    for i in range(2):
                        r = 4 * tt - 1 + 2 * j + i
                        if not (0 <= r < H):
                            continue
                        # xsc[(c,pxi)] = xhat[gbq, c, r, pxi]; 1.0 for c=3
                        xsc = np.ones((4, 32), np.float32)
                        for c in range(C):
                            xsc[c] = xx[gbq, c, r]
                        lstk[kk, i] = b6[2 * j + i] * xsc.reshape(128)[:, None]
        # -> [pi, k, pair, po]
        lstk = np.ascontiguousarray(lstk.transpose(2, 0, 1, 3)).reshape(128, -1)
        in_maps.append(
            {
                "rows": np.ascontiguousarray(rows.reshape(32, 128, F)).astype(f8),
                "lstk": lstk.astype(f8),
                "img16": np.ascontiguousarray(arr16),
                "img8": np.ascontiguousarray(arr8),
                "imgc2": np.ascontiguousarray(arrc2),
            }
        )

    import os

    trace = bool(os.environ.get("KERNEL_TRACE"))
    res = bass_utils.run_bass_kernel_spmd(
        nc, in_maps, core_ids=list(range(NCORES)), trace=trace
    )
    global _last_res
    _last_res = res

    sw = np.zeros((B, P), np.float64)
    swi = np.zeros((B, C, P), np.float64)
    for k in range(NCORES):
        bg = k // GN
        raw = np.asarray(res.results[k]["out"], np.float64)
        st = raw[:, : NB * PT * 4].reshape(128, NB, PT, 4)
        ex = raw[:, NB * PT * 4 :].reshape(128, NB, 4)
        st[:, :, 0, 0:3] += ex[:, :, 0:3]  # col 3 (c2) is never split
        for b in range(NB):
            gb = bg * NB + b
            sw[gb] += st[:, b, :, 0].T.reshape(P)
            for c in range(C):
                swi[gb, c] += st[:, b, :, 1 + c].T.reshape(P)

    score = (m * swi / sw[:, None, :] - x.reshape(B, C, P)) / sig2
    return score.reshape(B, C, H, W).astype(np.float32)


# revision 4
# speedup vs baseline: 1.0099x; 1.0065x over previous
"""LocalScoreMachine Trainium2 kernel, v4: fp8 DoubleRow box matmuls.

arg[b, po, n] = sum over 3 halo row-pairs of Lpair^T (x) rows-pair, where each
DoubleRow fp8 matmul contracts 2 row-ktiles (256 partitions) at 0.5 cyc/col.
Lpair matrices (band x xhat, fp8) are built on HOST and shipped (1.57MB),
removing all DVE L-builds. rows shipped fp8 (4.2MB, zero-padded border rows),
imgs f16 (6.3MB) for the DVE product+accum reductions. w = exp(arg) on ACT
(SW via accum_out); SWI_c = TT-mult (2x) + TSP accum (4x); a fraction of the
accums routes to ACT Copy+accum to balance DVE vs ACT.
Sharding: 4 query-pair groups x 2 dataset halves.
"""

import sys

for _p in ("/opt/trn_rl_repo", "/opt/trn_rl_repo/concourse", "/opt/pypackages"):
    if _p not in sys.path:
        sys.path.append(_p)

from contextlib import ExitStack

import numpy as np

import concourse.bass as bass
import concourse.bacc as bacc
import concourse.mybir as mybir
import concourse.tile as tile
from concourse import bass_utils

B, N, C, H, W = 8, 2048, 3, 32, 32
P = H * W
NCORES = 8
GB, GN = 4, 2
NB = B // GB  # 2
NLOC = N // GN  # 1024
PT = P // 128  # 8
F = NLOC
CH = 512
NPAIR = 3  # halo row-pairs per output tile

F32 = mybir.dt.float32
F16 = mybir.dt.float16
F8 = mybir.dt.float8e4
AF = mybir.ActivationFunctionType
OP = mybir.AluOpType
DR = mybir.MatmulPerfMode.DoubleRow

N_ACT = 9  # of the DVE-product accums, this many go to ACT Copy
POOL_NT = 7  # c=2 products run on Pool for t < POOL_NT (early, to avoid tail)

_cache = {}
_last_res = None


def _build():
    nc = bacc.Bacc("TRN2", target_bir_lowering=False, debug=False)

    rows_d = nc.dram_tensor("rows", [32, 128, F], F8, kind="ExternalInput")
    lstk_d = nc.dram_tensor("lstk", [128, 16 * NPAIR * 256], F8, kind="ExternalInput")
    img16_d = nc.dram_tensor("img16", [PT, 128, 2, F], F16, kind="ExternalInput")
    img8_d = nc.dram_tensor("img8", [POOL_NT, 128, F], F8, kind="ExternalInput")
    imgc2_d = nc.dram_tensor(
        "imgc2", [PT - POOL_NT, 128, F], F16, kind="ExternalInput"
    )
    out_d = nc.dram_tensor("out", [128, NB * PT * 4 + 8], F32, kind="ExternalOutput")

    with tile.TileContext(nc) as tc, ExitStack() as ctx:
        const = ctx.enter_context(tc.tile_pool(name="const", bufs=1))
        rpool = ctx.enter_context(tc.tile_pool(name="rpool", bufs=1))
        imgs = ctx.enter_context(tc.tile_pool(name="imgs", bufs=1))
        wp = ctx.enter_context(tc.tile_pool(name="wp", bufs=4))
        scr = ctx.enter_context(tc.tile_pool(name="scr", bufs=5))
        psum = ctx.enter_context(
            tc.tile_pool(name="psum", bufs=3, space=bass.MemorySpace.PSUM)
        )

        stage = const.tile([128, NB * PT * 4 + 8], F32)

        lstk = const.tile([128, 16 * NPAIR * 256], F8)
        rows = rpool.tile([128, 34 * F], F8)
        rows3 = rows[:].rearrange("p (r n) -> p r n", r=34)
        it16s, ic2s = [], []
        for g in range(PT):
            it16s.append(
                imgs.tile([128, 2 * F], F16, tag=f"img{g}", name=f"img{g}")
            )
            if g < POOL_NT:
                ic2s.append(imgs.tile([128, F], F8, tag=f"i8_{g}", name=f"i8_{g}"))
            else:
                ic2s.append(
                    imgs.tile([128, F], F16, tag=f"ic2_{g}", name=f"ic2_{g}")
                )

        # zero the border rows (0 and 33) on Pool instead of shipping them
        nc.gpsimd.memset(rows3[:, 0:1, :], 0)
        nc.gpsimd.memset(rows3[:, 33:34, :], 0)

        psz = NPAIR * 256  # lstk elems per step
        LCH = [(0, 2), (2, 4), (4, 7), (7, 12), (12, 16)]  # step ranges

        def dma_lstk(i):
            s0, s1 = LCH[i]
            nc.sync.dma_start(
                lstk[:, s0 * psz : s1 * psz], lstk_d.ap()[:, s0 * psz : s1 * psz]
            )

        # rows chunk ranges in DRAM row index r (sbuf index = r+1)
        RCH = [(0, 3), (3, 5), (5, 9), (9, 13), (13, 17), (17, 21),
               (21, 25), (25, 29), (29, 32)]

        def dma_rows(i):
            r0, r1 = RCH[i]
            nc.sync.dma_start(
                rows3[:, r0 + 1 : r1 + 1, :],
                rows_d.ap()[r0:r1].rearrange("r p n -> p r n"),
            )

        def dma_img(g):
            nc.sync.dma_start(
                it16s[g][:].rearrange("p (c n) -> p c n", c=2), img16_d.ap()[g]
            )
            if g < POOL_NT:
                nc.sync.dma_start(ic2s[g][:], img8_d.ap()[g])
            else:
                nc.sync.dma_start(ic2s[g][:], imgc2_d.ap()[g - POOL_NT])

        # single queue, earliest-deadline order: the DMA_ENGINES device
        # serializes transfers in ready order, so issue order == arrival order
        dma_lstk(0); dma_rows(0); dma_rows(1); dma_img(0)
        dma_lstk(1); dma_rows(2); dma_img(1)
        dma_lstk(2); dma_rows(3); dma_img(2)
        dma_rows(4); dma_lstk(3); dma_img(3)
        dma_rows(5); dma_img(4)
        dma_rows(6); dma_lstk(4); dma_img(5)
        dma_rows(7); dma_img(6)
        dma_rows(8); dma_img(7)

        def scol(b, t, j):
            k = b * (PT * 4) + t * 4 + j
            return stage[:, k : k + 1]

        def scol2(b, j):  # extra columns for the t=0 second-half partials
            k = NB * PT * 4 + b * 4 + j
            return stage[:, k : k + 1]

        def lpair_ap(b, t, j):
            k = ((t * NB + b) * NPAIR + j) * 256
            return lstk[:, k : k + 256].rearrange("p (two m) -> p two m", two=2)

        pst = [[None] * PT for _ in range(NB)]

        def emit_mm(b, t):
            # t=0 uses two half psum tiles so each half-exp can start as soon
            # as its own 3 matmuls finish (psum deps are tile-granular)
            if t == 0:
                hs = []
                for ck in range(F // CH):
                    ph = psum.tile(
                        [128, CH], F32, tag=f"psh{ck}", name=f"psh_{b}_{ck}",
                        bufs=1,
                    )
                    for j in range(NPAIR):
                        ri0 = 4 * t + 2 * j
                        nc.tensor.matmul(
                            ph[:],
                            lpair_ap(b, t, j),
                            rows3[:, ri0 : ri0 + 2, ck * CH : (ck + 1) * CH],
                            start=(j == 0),
                            stop=(j == NPAIR - 1),
                            perf_mode=DR,
                            skip_group_check=True,
                        )
                    hs.append(ph)
                pst[b][t] = hs
                return
            ps = psum.tile([128, F], F32, tag="ps", name=f"ps_{b}_{t}")
            for ck in range(F // CH):
                for j in range(NPAIR):
                    ri0 = 4 * t + 2 * j  # rows index of pair start (r = ri0-1)
                    nc.tensor.matmul(
                        ps[:, ck * CH : (ck + 1) * CH],
                        lpair_ap(b, t, j),
                        rows3[:, ri0 : ri0 + 2, ck * CH : (ck + 1) * CH],
                        start=(j == 0),
                        stop=(j == NPAIR - 1),
                        perf_mode=DR,
                        skip_group_check=True,
                    )
            pst[b][t] = ps

        wts = [[None] * PT for _ in range(NB)]

        def emit_exp(b, t):
            w = wp.tile([128, F], F16, tag="w", name=f"w_{b}_{t}")
            ps = pst[b][t]
            if t == 0:
                nc.scalar.activation(
                    w[:, 0:CH], ps[0][:], AF.Exp, accum_out=scol(b, t, 0)
                )
                nc.scalar.activation(
                    w[:, CH:F], ps[1][:], AF.Exp, accum_out=scol2(b, 0)
                )
            else:
                nc.scalar.activation(
                    w[:], ps[:], AF.Exp, accum_out=scol(b, t, 0)
                )
            wts[b][t] = w

        aq = [0.0]
        n_dve_accum_cands = 4 * 6

        def accum(v_slice, b, t, c, early=True):
            if early and t < 6:
                aq[0] += N_ACT / float(n_dve_accum_cands)
            if aq[0] >= 1.0:
                aq[0] -= 1.0
                d = scr.tile([128, F], F16, tag="da", name=f"d{c}_{b}_{t}")
                nc.scalar.activation(
                    d[:], v_slice, AF.Copy, accum_out=scol(b, t, 1 + c)
                )
            else:
                d = scr.tile([128, F], F16, tag="dv", name=f"d{c}_{b}_{t}")
                nc.vector.tensor_scalar(
                    d[:], v_slice, 1.0, 0.0, OP.mult, OP.add,
                    accum_out=scol(b, t, 1 + c),
                )


        pend = []

        def flush_pend(keep=0):
            while len(pend) > keep:
                v1p, bp, tp = pend.pop(0)
                d = scr.tile([128, F], F16, tag="dv", name=f"d2_{bp}_{tp}")
                nc.vector.tensor_scalar(
                    d[:], v1p[:], 1.0, 0.0, OP.mult, OP.add,
                    accum_out=scol(bp, tp, 3),
                )

        def emit_red(b, t):
            w = wts[b][t]
            it16r = it16s[t][:].rearrange("p (c n) -> p c n", c=2)
            if t == 0:
                # two half-width passes: the first starts as soon as the
                # first half-exp lands
                v2 = scr.tile([128, 2 * F], F16, tag="v2", name=f"v2_{b}_{t}")
                v2r = v2[:].rearrange("p (c n) -> p c n", c=2)
                for h, (n0, n1) in enumerate([(0, CH), (CH, F)]):
                    nc.vector.tensor_tensor(
                        v2r[:, :, n0:n1],
                        w[:, n0:n1].unsqueeze(1).broadcast_to((128, 2, CH)),
                        it16r[:, :, n0:n1],
                        OP.mult,
                    )
                    for c in range(2):
                        col = scol(b, t, 1 + c) if h == 0 else scol2(b, 1 + c)
                        d = scr.tile(
                            [128, CH], F16, tag="dv2", name=f"e{c}_{b}_{h}"
                        )
                        nc.vector.tensor_scalar(
                            d[:], v2r[:, c, n0:n1], 1.0, 0.0, OP.mult, OP.add,
                            accum_out=col,
                        )
                v1 = scr.tile([128, F], F16, tag="vp", name=f"v1_{b}_{t}")
                nc.gpsimd.tensor_tensor(v1[:], w[:], ic2s[t][:], OP.mult)
                pend.append((v1, b, t))
                return
            # c=0,1 in one 2-wide DVE op: out[(c, n)]
            v2 = scr.tile([128, 2 * F], F16, tag="v2", name=f"v2_{b}_{t}")
            v2r = v2[:].rearrange("p (c n) -> p c n", c=2)
            nc.vector.tensor_tensor(
                v2r,
                w[:].unsqueeze(1).broadcast_to((128, 2, F)),
                it16r,
                OP.mult,
            )
            # c=2: Pool (fp8 images) for t < POOL_NT, else DVE (f16 images);
            # the final step stays on DVE so the tail isn't gated on Pool
            if t < POOL_NT or (t == PT - 1 and b == 0):
                v1 = scr.tile([128, F], F16, tag="vp", name=f"v1_{b}_{t}")
                nc.gpsimd.tensor_tensor(v1[:], w[:], ic2s[t][:], OP.mult)
                pend.append((v1, b, t))
            else:
                v1 = scr.tile([128, F], F16, tag="vp", name=f"v1_{b}_{t}")
                nc.vector.tensor_tensor(v1[:], w[:], ic2s[t][:], OP.mult)
            for c in range(2):
                if t == PT - 2 and b == NB - 1 and c == 1:
                    d = scr.tile([128, F], F16, tag="da", name="dl61")
                    nc.scalar.activation(
                        d[:], v2r[:, c, :], AF.Copy, accum_out=scol(b, t, 1 + c)
                    )
                    continue
                if t == PT - 1 and b == NB - 1 and c == 0:
                    d = scr.tile([128, F], F16, tag="da", name=f"dl{c}")
                    nc.scalar.activation(
                        d[:], v2r[:, c, :], AF.Copy, accum_out=scol(b, t, 1 + c)
                    )
                else:
                    accum(v2r[:, c, :], b, t, c)
            flush_pend(keep=1)
            if t >= POOL_NT:
                accum(v1[:], b, t, 2)

        steps = [(t, b) for t in range(PT) for b in range(NB)]
        nst = len(steps)
        for k in range(nst + 4):
            if k < nst:
                t, b = steps[k]
                emit_mm(b, t)
            if 2 <= k < nst + 2:
                t1, b1 = steps[k - 2]
                emit_exp(b1, t1)
            if 4 <= k < nst + 4:
                t2, b2 = steps[k - 4]
                emit_red(b2, t2)

        flush_pend()
        nc.sync.dma_start(out_d.ap(), stage[:])

    nc.compile()
    return nc


def _band6():
    # b6[j][(c,px_in), po]: band for input row-offset j-1 vs output row po//32
    px_i = np.arange(128) % 32
    po = np.arange(128)
    ro, pxo = po // 32, po % 32
    b6 = np.zeros((6, 128, 128), np.float32)
    for j in range(6):
        rd = (j - 1) - ro[None, :]
        wd = px_i[:, None] - pxo[None, :]
        b6[j] = ((np.abs(rd) <= 1) & (np.abs(wd) <= 1)).astype(np.float32)
    return b6


def kernel(x, images, mu, sigma, t):
    x = np.ascontiguousarray(np.asarray(x, dtype=np.float32))
    images = np.ascontiguousarray(np.asarray(images, dtype=np.float32))
    m = float(np.asarray(mu)[int(t)])
    sig = float(np.asarray(sigma)[int(t)])
    sig2 = sig * sig

    if "nc" not in _cache:
        _cache["nc"] = _build()
    nc = _cache["nc"]

    f8 = mybir.dt.np(F8)
    imgs2 = images.reshape(N, C, P)
    spp = (-(m * m) / (2.0 * sig2)) * np.einsum("ncp,ncp->np", imgs2, imgs2)
    xx = (x.reshape(B, C, H, W) * (m / sig2)).astype(np.float32)
    b6 = _band6()

    in_maps = []
    for k in range(NCORES):
        bg, nh = k // GN, k % GN
        nsl = slice(nh * NLOC, (nh + 1) * NLOC)
        # rows[r]: [(c,px), n]; c<3 = I_c row r, c=3 = S'' row r
        rows = np.zeros((32, 4, 32, F), np.float32)
        for c in range(C):
            rows[:, c] = imgs2[nsl, c, :].T.reshape(H, 32, F)
        rows[:, 3] = spp[nsl].T.reshape(H, 32, F)
        # imgs[t]: [(4r x 32px), c, n] for the product reductions; c=0,1 f16,
        # c=2 fp8 for the Pool tiles (t < POOL_NT) and f16 for the rest
        arr16 = np.empty((PT, 128, 2, F), np.float16)
        for c in range(2):
            arr16[:, :, c, :] = imgs2[nsl, c, :].T.reshape(PT, 128, F)
        c2 = imgs2[nsl, 2, :].T.reshape(PT, 128, F)
        arr8 = c2[:POOL_NT].astype(f8)
        arrc2 = c2[POOL_NT:].astype(np.float16)
        # L pair stacks: [(c,pxi), (t,b,j,pair,po)] fp8
        lstk = np.zeros((16 * NPAIR, 2, 128, 128), np.float32)  # [k,pair,pi,po]
        for tt in range(PT):
            for b in range(NB):
                gbq = bg * NB + b
                for j in range(NPAIR):
                    kk = (tt * NB + b) * NPAIR + j
                    for i in range(2):
                        r = 4 * tt - 1 + 2 * j + i
                        if not (0 <= r < H):
                            continue
                        # xsc[(c,pxi)] = xhat[gbq, c, r, pxi]; 1.0 for c=3
                        xsc = np.ones((4, 32), np.float32)
                        for c in range(C):
                            xsc[c] = xx[gbq, c, r]
                        lstk[kk, i] = b6[2 * j + i] * xsc.reshape(128)[:, None]
        # -> [pi, k, pair, po]
        lstk = np.ascontiguousarray(lstk.transpose(2, 0, 1, 3)).reshape(128, -1)
        in_maps.append(
            {
                "rows": np.ascontiguousarray(rows.reshape(32, 128, F)).astype(f8),
                "lstk": lstk.astype(f8),
                "img16": np.ascontiguousarray(arr16),
                "img8": np.ascontiguousarray(arr8),
                "imgc2": np.ascontiguousarray(arrc2),
            }
        )

    import os

    trace = bool(os.environ.get("KERNEL_TRACE"))
    res = bass_utils.run_bass_kernel_spmd(
        nc, in_maps, core_ids=list(range(NCORES)), trace=trace
    )
    global _last_res
    _last_res = res

    sw = np.zeros((B, P), np.float64)
    swi = np.zeros((B, C, P), np.float64)
    for k in range(NCORES):
        bg = k // GN
        raw = np.asarray(res.results[k]["out"], np.float64)
        st = raw[:, : NB * PT * 4].reshape(128, NB, PT, 4)
        ex = raw[:, NB * PT * 4 :].reshape(128, NB, 4)
        st[:, :, 0, 0:3] += ex[:, :, 0:3]  # col 3 (c2) is never split
        for b in range(NB):
            gb = bg * NB + b
            sw[gb] += st[:, b, :, 0].T.reshape(P)
            for c in range(C):
                swi[gb, c] += st[:, b, :, 1 + c].T.reshape(P)

    score = (m * swi / sw[:, None, :] - x.reshape(B, C, P)) / sig2
    return score.reshape(B, C, H, W).astype(np.float32)


# revision 5
# speedup vs baseline: 1.0159x; 1.0060x over previous
"""LocalScoreMachine Trainium2 kernel, v4: fp8 DoubleRow box matmuls.

arg[b, po, n] = sum over 3 halo row-pairs of Lpair^T (x) rows-pair, where each
DoubleRow fp8 matmul contracts 2 row-ktiles (256 partitions) at 0.5 cyc/col.
Lpair matrices (band x xhat, fp8) are built on HOST and shipped (1.57MB),
removing all DVE L-builds. rows shipped fp8 (4.2MB, zero-padded border rows),
imgs f16 (6.3MB) for the DVE product+accum reductions. w = exp(arg) on ACT
(SW via accum_out); SWI_c = TT-mult (2x) + TSP accum (4x); a fraction of the
accums routes to ACT Copy+accum to balance DVE vs ACT.
Sharding: 4 query-pair groups x 2 dataset halves.
"""

import sys

for _p in ("/opt/trn_rl_repo", "/opt/trn_rl_repo/concourse", "/opt/pypackages"):
    if _p not in sys.path:
        sys.path.append(_p)

from contextlib import ExitStack

import numpy as np

import concourse.bass as bass
import concourse.bacc as bacc
import concourse.mybir as mybir
import concourse.tile as tile
from concourse import bass_utils

B, N, C, H, W = 8, 2048, 3, 32, 32
P = H * W
NCORES = 8
GB, GN = 4, 2
NB = B // GB  # 2
NLOC = N // GN  # 1024
PT = P // 128  # 8
F = NLOC
CH = 512
NPAIR = 3  # halo row-pairs per output tile

F32 = mybir.dt.float32
F16 = mybir.dt.float16
F8 = mybir.dt.float8e4
AF = mybir.ActivationFunctionType
OP = mybir.AluOpType
DR = mybir.MatmulPerfMode.DoubleRow

N_ACT = 9  # of the DVE-product accums, this many go to ACT Copy
POOL_NT = 7  # c=2 products run on Pool for t < POOL_NT (early, to avoid tail)

_cache = {}
_last_res = None


def _build():
    nc = bacc.Bacc("TRN2", target_bir_lowering=False, debug=False)

    rows_d = nc.dram_tensor("rows", [32, 128, F], F8, kind="ExternalInput")
    lstk_d = nc.dram_tensor("lstk", [128, 16 * NPAIR * 256], F8, kind="ExternalInput")
    img16_d = nc.dram_tensor("img16", [PT, 128, 2, F], F16, kind="ExternalInput")
    img8_d = nc.dram_tensor("img8", [POOL_NT, 128, F], F8, kind="ExternalInput")
    imgc2_d = nc.dram_tensor(
        "imgc2", [PT - POOL_NT, 128, F], F16, kind="ExternalInput"
    )
    out_d = nc.dram_tensor("out", [128, NB * PT * 4 + 8], F32, kind="ExternalOutput")

    with tile.TileContext(nc) as tc, ExitStack() as ctx:
        const = ctx.enter_context(tc.tile_pool(name="const", bufs=1))
        rpool = ctx.enter_context(tc.tile_pool(name="rpool", bufs=1))
        imgs = ctx.enter_context(tc.tile_pool(name="imgs", bufs=1))
        wp = ctx.enter_context(tc.tile_pool(name="wp", bufs=4))
        scr = ctx.enter_context(tc.tile_pool(name="scr", bufs=5))
        psum = ctx.enter_context(
            tc.tile_pool(name="psum", bufs=3, space=bass.MemorySpace.PSUM)
        )

        stage = const.tile([128, NB * PT * 4 + 8], F32)

        lstk = const.tile([128, 16 * NPAIR * 256], F8)
        rows = rpool.tile([128, 34 * F], F8)
        rows3 = rows[:].rearrange("p (r n) -> p r n", r=34)
        it16s, ic2s = [], []
        for g in range(PT):
            it16s.append(
                imgs.tile([128, 2 * F], F16, tag=f"img{g}", name=f"img{g}")
            )
            if g < POOL_NT:
                ic2s.append(imgs.tile([128, F], F8, tag=f"i8_{g}", name=f"i8_{g}"))
            else:
                ic2s.append(
                    imgs.tile([128, F], F16, tag=f"ic2_{g}", name=f"ic2_{g}")
                )

        # zero the border rows (0 and 33) on Pool instead of shipping them
        nc.gpsimd.memset(rows3[:, 0:1, :], 0)
        nc.gpsimd.memset(rows3[:, 33:34, :], 0)

        psz = NPAIR * 256  # lstk elems per step
        LCH = [(0, 2), (2, 4), (4, 7), (7, 12), (12, 16)]  # step ranges

        def dma_lstk(i):
            s0, s1 = LCH[i]
            nc.sync.dma_start(
                lstk[:, s0 * psz : s1 * psz], lstk_d.ap()[:, s0 * psz : s1 * psz]
            )

        # rows chunk ranges in DRAM row index r (sbuf index = r+1)
        RCH = [(0, 3), (3, 5), (5, 9), (9, 13), (13, 17), (17, 21),
               (21, 25), (25, 29), (29, 32)]

        def dma_rows(i):
            r0, r1 = RCH[i]
            nc.sync.dma_start(
                rows3[:, r0 + 1 : r1 + 1, :],
                rows_d.ap()[r0:r1].rearrange("r p n -> p r n"),
            )

        def dma_img(g):
            nc.sync.dma_start(
                it16s[g][:].rearrange("p (c n) -> p c n", c=2), img16_d.ap()[g]
            )
            if g < POOL_NT:
                nc.sync.dma_start(ic2s[g][:], img8_d.ap()[g])
            else:
                nc.sync.dma_start(ic2s[g][:], imgc2_d.ap()[g - POOL_NT])

        # single queue, earliest-deadline order: the DMA_ENGINES device
        # serializes transfers in ready order, so issue order == arrival order
        dma_lstk(0); dma_rows(0); dma_rows(1); dma_img(0)
        dma_lstk(1); dma_rows(2); dma_img(1)
        dma_lstk(2); dma_rows(3); dma_img(2)
        dma_rows(4); dma_lstk(3); dma_img(3)
        dma_rows(5); dma_img(4)
        dma_rows(6); dma_lstk(4); dma_img(5)
        dma_rows(7); dma_img(6)
        dma_rows(8); dma_img(7)

        def scol(b, t, j):
            k = b * (PT * 4) + t * 4 + j
            return stage[:, k : k + 1]

        def scol2(b, j):  # extra columns for the t=0 second-half partials
            k = NB * PT * 4 + b * 4 + j
            return stage[:, k : k + 1]

        def lpair_ap(b, t, j):
            k = ((t * NB + b) * NPAIR + j) * 256
            return lstk[:, k : k + 256].rearrange("p (two m) -> p two m", two=2)

        pst = [[None] * PT for _ in range(NB)]

        def emit_mm(b, t):
            # t=0 uses two half psum tiles so each half-exp can start as soon
            # as its own 3 matmuls finish (psum deps are tile-granular)
            if t == 0:
                hs = []
                for ck in range(F // CH):
                    ph = psum.tile(
                        [128, CH], F32, tag=f"psh{ck}", name=f"psh_{b}_{ck}",
                        bufs=1,
                    )
                    for j in range(NPAIR):
                        ri0 = 4 * t + 2 * j
                        nc.tensor.matmul(
                            ph[:],
                            lpair_ap(b, t, j),
                            rows3[:, ri0 : ri0 + 2, ck * CH : (ck + 1) * CH],
                            start=(j == 0),
                            stop=(j == NPAIR - 1),
                            perf_mode=DR,
                            skip_group_check=True,
                        )
                    hs.append(ph)
                pst[b][t] = hs
                return
            ps = psum.tile([128, F], F32, tag="ps", name=f"ps_{b}_{t}")
            for ck in range(F // CH):
                for j in range(NPAIR):
                    ri0 = 4 * t + 2 * j  # rows index of pair start (r = ri0-1)
                    nc.tensor.matmul(
                        ps[:, ck * CH : (ck + 1) * CH],
                        lpair_ap(b, t, j),
                        rows3[:, ri0 : ri0 + 2, ck * CH : (ck + 1) * CH],
                        start=(j == 0),
                        stop=(j == NPAIR - 1),
                        perf_mode=DR,
                        skip_group_check=True,
                    )
            pst[b][t] = ps

        wts = [[None] * PT for _ in range(NB)]

        def emit_exp(b, t):
            w = wp.tile([128, F], F16, tag="w", name=f"w_{b}_{t}")
            ps = pst[b][t]
            if t == 0:
                nc.scalar.activation(
                    w[:, 0:CH], ps[0][:], AF.Exp, accum_out=scol(b, t, 0)
                )
                nc.scalar.activation(
                    w[:, CH:F], ps[1][:], AF.Exp, accum_out=scol2(b, 0)
                )
            else:
                nc.scalar.activation(
                    w[:], ps[:], AF.Exp, accum_out=scol(b, t, 0)
                )
            wts[b][t] = w

        aq = [0.0]
        n_dve_accum_cands = 4 * 6

        def accum(v_slice, b, t, c, early=True):
            if early and t < 6:
                aq[0] += N_ACT / float(n_dve_accum_cands)
            if aq[0] >= 1.0:
                aq[0] -= 1.0
                d = scr.tile([128, F], F16, tag="da", name=f"d{c}_{b}_{t}")
                nc.scalar.activation(
                    d[:], v_slice, AF.Copy, accum_out=scol(b, t, 1 + c)
                )
            else:
                d = scr.tile([128, F], F16, tag="dv", name=f"d{c}_{b}_{t}")
                nc.vector.tensor_scalar(
                    d[:], v_slice, 1.0, 0.0, OP.mult, OP.add,
                    accum_out=scol(b, t, 1 + c),
                )


        pend = []

        def flush_pend(keep=0):
            while len(pend) > keep:
                v1p, bp, tp = pend.pop(0)
                d = scr.tile([128, F], F16, tag="dv", name=f"d2_{bp}_{tp}")
                nc.vector.tensor_scalar(
                    d[:], v1p[:], 1.0, 0.0, OP.mult, OP.add,
                    accum_out=scol(bp, tp, 3),
                )

        def emit_red(b, t):
            w = wts[b][t]
            it16r = it16s[t][:].rearrange("p (c n) -> p c n", c=2)
            if t == 0:
                # two half-width passes: the first starts as soon as the
                # first half-exp lands
                v2 = scr.tile([128, 2 * F], F16, tag="v2", name=f"v2_{b}_{t}")
                v2r = v2[:].rearrange("p (c n) -> p c n", c=2)
                for h, (n0, n1) in enumerate([(0, CH), (CH, F)]):
                    nc.vector.tensor_tensor(
                        v2r[:, :, n0:n1],
                        w[:, n0:n1].unsqueeze(1).broadcast_to((128, 2, CH)),
                        it16r[:, :, n0:n1],
                        OP.mult,
                    )
                    for c in range(2):
                        col = scol(b, t, 1 + c) if h == 0 else scol2(b, 1 + c)
                        d = scr.tile(
                            [128, CH], F16, tag="dv2", name=f"e{c}_{b}_{h}"
                        )
                        nc.vector.tensor_scalar(
                            d[:], v2r[:, c, n0:n1], 1.0, 0.0, OP.mult, OP.add,
                            accum_out=col,
                        )
                v1 = scr.tile([128, F], F16, tag="vp", name=f"v1_{b}_{t}")
                nc.gpsimd.tensor_tensor(v1[:], w[:], ic2s[t][:], OP.mult)
                pend.append((v1, b, t))
                return
            # c=0,1 in one 2-wide DVE op: out[(c, n)]
            v2 = scr.tile([128, 2 * F], F16, tag="v2", name=f"v2_{b}_{t}")
            v2r = v2[:].rearrange("p (c n) -> p c n", c=2)
            nc.vector.tensor_tensor(
                v2r,
                w[:].unsqueeze(1).broadcast_to((128, 2, F)),
                it16r,
                OP.mult,
            )
            # c=2: Pool (fp8 images) for t < POOL_NT, else DVE (f16 images);
            # the final step stays on DVE so the tail isn't gated on Pool
            if t < POOL_NT or (t == PT - 1 and b == 0):
                v1 = scr.tile([128, F], F16, tag="vp", name=f"v1_{b}_{t}")
                nc.gpsimd.tensor_tensor(v1[:], w[:], ic2s[t][:], OP.mult)
                pend.append((v1, b, t))
            else:
                v1 = scr.tile([128, F], F16, tag="vp", name=f"v1_{b}_{t}")
                nc.vector.tensor_tensor(v1[:], w[:], ic2s[t][:], OP.mult)
            for c in range(2):
                if t == PT - 2 and b == NB - 1 and c == 1:
                    d = scr.tile([128, F], F16, tag="da", name="dl61")
                    nc.scalar.activation(
                        d[:], v2r[:, c, :], AF.Copy, accum_out=scol(b, t, 1 + c)
                    )
                    continue
                if t == PT - 1 and b == NB - 1 and c == 0:
                    d = scr.tile([128, F], F16, tag="da", name=f"dl{c}")
                    nc.scalar.activation(
                        d[:], v2r[:, c, :], AF.Copy, accum_out=scol(b, t, 1 + c)
                    )
                else:
                    accum(v2r[:, c, :], b, t, c)
            flush_pend(keep=1)
            if t >= POOL_NT:
                accum(v1[:], b, t, 2)

        steps = [(t, b) for t in range(PT) for b in range(NB)]
        nst = len(steps)
        for k in range(nst + 4):
            if k < nst:
                t, b = steps[k]
                emit_mm(b, t)
            if 2 <= k < nst + 2:
                t1, b1 = steps[k - 2]
                emit_exp(b1, t1)
            if 4 <= k < nst + 4:
                t2, b2 = steps[k - 4]
                emit_red(b2, t2)

        # drain remaining deferred pool accums via ACT (idle at the tail,
        # keeps the last DVE op earlier)
        while pend:
            v1p, bp, tp = pend.pop(0)
            da = scr.tile([128, F], F16, tag="da", name=f"dp_{bp}_{tp}")
            nc.scalar.activation(
                da[:], v1p[:], AF.Copy, accum_out=scol(bp, tp, 3)
            )
        nc.sync.dma_start(out_d.ap(), stage[:])

    nc.compile()
    return nc


def _band6():
    # b6[j][(c,px_in), po]: band for input row-offset j-1 vs output row po//32
    px_i = np.arange(128) % 32
    po = np.arange(128)
    ro, pxo = po // 32, po % 32
    b6 = np.zeros((6, 128, 128), np.float32)
    for j in range(6):
        rd = (j - 1) - ro[None, :]
        wd = px_i[:, None] - pxo[None, :]
        b6[j] = ((np.abs(rd) <= 1) & (np.abs(wd) <= 1)).astype(np.float32)
    return b6


def kernel(x, images, mu, sigma, t):
    x = np.ascontiguousarray(np.asarray(x, dtype=np.float32))
    images = np.ascontiguousarray(np.asarray(images, dtype=np.float32))
    m = float(np.asarray(mu)[int(t)])
    sig = float(np.asarray(sigma)[int(t)])
    sig2 = sig * sig

    if "nc" not in _cache:
        _cache["nc"] = _build()
    nc = _cache["nc"]

    f8 = mybir.dt.np(F8)
    imgs2 = images.reshape(N, C, P)
    spp = (-(m * m) / (2.0 * sig2)) * np.einsum("ncp,ncp->np", imgs2, imgs2)
    xx = (x.reshape(B, C, H, W) * (m / sig2)).astype(np.float32)
    b6 = _band6()

    in_maps = []
    for k in range(NCORES):
        bg, nh = k // GN, k % GN
        nsl = slice(nh * NLOC, (nh + 1) * NLOC)
        # rows[r]: [(c,px), n]; c<3 = I_c row r, c=3 = S'' row r
        rows = np.zeros((32, 4, 32, F), np.float32)
        for c in range(C):
            rows[:, c] = imgs2[nsl, c, :].T.reshape(H, 32, F)
        rows[:, 3] = spp[nsl].T.reshape(H, 32, F)
        # imgs[t]: [(4r x 32px), c, n] for the product reductions; c=0,1 f16,
        # c=2 fp8 for the Pool tiles (t < POOL_NT) and f16 for the rest
        arr16 = np.empty((PT, 128, 2, F), np.float16)
        for c in range(2):
            arr16[:, :, c, :] = imgs2[nsl, c, :].T.reshape(PT, 128, F)
        c2 = imgs2[nsl, 2, :].T.reshape(PT, 128, F)
        arr8 = c2[:POOL_NT].astype(f8)
        arrc2 = c2[POOL_NT:].astype(np.float16)
        # L pair stacks: [(c,pxi), (t,b,j,pair,po)] fp8
        lstk = np.zeros((16 * NPAIR, 2, 128, 128), np.float32)  # [k,pair,pi,po]
        for tt in range(PT):
            for b in range(NB):
                gbq = bg * NB + b
                for j in range(NPAIR):
                    kk = (tt * NB + b) * NPAIR + j
                    for i in range(2):
                        r = 4 * tt - 1 + 2 * j + i
                        if not (0 <= r < H):
                            continue
                        # xsc[(c,pxi)] = xhat[gbq, c, r, pxi]; 1.0 for c=3
                        xsc = np.ones((4, 32), np.float32)
                        for c in range(C):
                            xsc[c] = xx[gbq, c, r]
                        lstk[kk, i] = b6[2 * j + i] * xsc.reshape(128)[:, None]
        # -> [pi, k, pair, po]
        lstk = np.ascontiguousarray(lstk.transpose(2, 0, 1, 3)).reshape(128, -1)
        in_maps.append(
            {
                "rows": np.ascontiguousarray(rows.reshape(32, 128, F)).astype(f8),
                "lstk": lstk.astype(f8),
                "img16": np.ascontiguousarray(arr16),
                "img8": np.ascontiguousarray(arr8),
                "imgc2": np.ascontiguousarray(arrc2),
            }
        )

    import os

    trace = bool(os.environ.get("KERNEL_TRACE"))
    res = bass_utils.run_bass_kernel_spmd(
        nc, in_maps, core_ids=list(range(NCORES)), trace=trace
    )
    global _last_res
    _last_res = res

    sw = np.zeros((B, P), np.float64)
    swi = np.zeros((B, C, P), np.float64)
    for k in range(NCORES):
        bg = k // GN
        raw = np.asarray(res.results[k]["out"], np.float64)
        st = raw[:, : NB * PT * 4].reshape(128, NB, PT, 4)
        ex = raw[:, NB * PT * 4 :].reshape(128, NB, 4)
        st[:, :, 0, 0:3] += ex[:, :, 0:3]  # col 3 (c2) is never split
        for b in range(NB):
            gb = bg * NB + b
            sw[gb] += st[:, b, :, 0].T.reshape(P)
            for c in range(C):
                swi[gb, c] += st[:, b, :, 1 + c].T.reshape(P)

    score = (m * swi / sw[:, None, :] - x.reshape(B, C, P)) / sig2
    return score.reshape(B, C, H, W).astype(np.float32)
